# revision 2
# baseline (speedup 1.0000x reference)
"""Causal multi-head attention (RoPE) on 8 TRN2 NeuronCores.

Sharding: Megatron-style head parallelism. Each core owns 2 of the 16 heads:
it computes q/k/v projections for its 128 output features (2 heads x 64),
applies interleaved-pair RoPE (rotation done as a PE matmul with a constant
pair-swap matrix R, tables precomputed host-side), runs causal attention for
its (batch, head) pairs in the transposed orientation S^T = K^T Q so that no
on-chip transposes of the attention matrix are needed, and accumulates
attn^T-weighted V with an extra all-ones V column that yields the softmax
denominators for free. An AllToAll then redistributes the per-head outputs
from head-sharded to token-sharded layout, and each core computes the final
output projection for its 512-token slice. A tiny dummy AllToAll is issued at
kernel start to absorb the one-time collective warmup (~95us) while the
engines compute.

Compute dtype: float32r (TF32-like full-rate fp32 matmul path; ~2^-11
element rounding, accumulation in fp32 PSUM).
"""

import sys

sys.path.insert(0, "/opt/trn_rl_repo")

import numpy as np

B, L, D, N, H = 2, 2048, 1024, 16, 64
T = B * L            # 4096 tokens, batch-major
NC = 8               # cores
HPC = N // NC        # 2 heads per core
W = HPC * H          # 128 projection features per core
THETA = 10000.0
VBLK = 130           # v_sb block stride: [h0 64 | ones 1 | h1 64 | ones 1]
NEG = -60.0          # additive causal mask value (exp(-60) ~ 9e-27)

_CACHED = {}


def _build():
    import concourse.bass as bass
    import concourse.mybir as mybir
    import concourse.tile as tile

    F32 = mybir.dt.float32
    F32R = mybir.dt.float32r
    AF = mybir.ActivationFunctionType

    # ---- fix 1: stock _drain_and_barrier overflows the 2-slot sync encoding
    import re as _re
    from concourse.vector_clock import ScopedClock, VectorClock

    def _split_drain_and_barrier(self, tick_clock, wait_clock):
        gc = tick_clock.global_clock
        ticks = [int(v) for v in _re.findall(r"-?\d+", str(gc))]
        for proc, t in enumerate(ticks):
            if t <= 0:
                continue
            sub = VectorClock()
            sub.require_at_least(proc, t)
            d = self.nc.sync.drain()
            wait_clock.add_sem_waits(d.ins, ScopedClock({None: sub}))
        self.nc.all_engine_barrier()
        assert self.sems is not None
        popped = self.nc._tile_sem_poison_stack.pop()
        assert popped is self._sem_poison
        self.nc.clear_and_free_semaphores(list(self.sems.allocated().values()))
        self.nc.all_engine_barrier()

    tile.TileContext._drain_and_barrier = _split_drain_and_barrier

    nc = bass.Bass()

    xT_ext = nc.declare_dram_parameter("xT", [D, T], F32R, isOutput=False)
    wq_ext = nc.declare_dram_parameter("wq", [W, D], F32R, isOutput=False)
    wk_ext = nc.declare_dram_parameter("wk", [W, D], F32R, isOutput=False)
    wv_ext = nc.declare_dram_parameter("wv", [W, D], F32R, isOutput=False)
    bq_ext = nc.declare_dram_parameter("bq", [W, 1], F32, isOutput=False)
    bk_ext = nc.declare_dram_parameter("bk", [W, 1], F32, isOutput=False)
    bv_ext = nc.declare_dram_parameter("bv", [W, 1], F32, isOutput=False)
    cos_ext = nc.declare_dram_parameter("cosT", [W, L], F32R, isOutput=False)
    sin_ext = nc.declare_dram_parameter("sinT", [W, L], F32R, isOutput=False)
    rmat_ext = nc.declare_dram_parameter("rmat", [128, 128], F32R, isOutput=False)
    ident_ext = nc.declare_dram_parameter("ident", [128, 128], F32R, isOutput=False)
    mask_ext = nc.declare_dram_parameter("masks", [128, 2048], F32, isOutput=False)
    ones_ext = nc.declare_dram_parameter("onesc", [128, 128], F32R, isOutput=False)
    wo_ext = nc.declare_dram_parameter("woT", [128, NC * D], F32R, isOutput=False)
    bo_ext = nc.declare_dram_parameter("bo", [1, D], F32R, isOutput=False)
    out_ext = nc.declare_dram_parameter("out", [T // NC, D], F32, isOutput=True)

    TT = T // 512      # 8 token tiles of 512
    KD = D // 128      # 8 contraction chunks

    with tile.TileContext(nc) as tc, nc.allow_low_precision(reason="f32r attention"):
        with tc.tile_pool(name="dram", bufs=1, space="DRAM") as dram:
            # dummy collective: absorbs the one-time collective warmup cost
            # while phase 1/2 compute runs. Data content is irrelevant.
            dum_in = dram.tile([NC, 128], F32)
            dum_out = dram.tile([NC, 128], F32)
            nc.gpsimd.collective_compute(
                "AllToAll", mybir.AluOpType.bypass,
                replica_groups=[list(range(NC))],
                ins=[dum_in[:].opt()], outs=[dum_out[:].opt()],
            )

            cin = dram.tile([NC, 128, 512], F32R)
            cout = dram.tile([NC, 128, 512], F32R)

            with tc.tile_pool(name="const", bufs=1) as cpool, \
                 tc.tile_pool(name="obuf", bufs=1) as opool:
                # constants
                wq_sb = cpool.tile([128, KD * 128], F32R)
                wk_sb = cpool.tile([128, KD * 128], F32R)
                wv_sb = cpool.tile([128, KD * 128], F32R)
                for k in range(KD):
                    # host layout: w[W, D]; lhsT chunk k = w[:, k*128:(k+1)*128].T
                    nc.sync.dma_start(
                        wq_sb[:, k * 128:(k + 1) * 128],
                        wq_ext[:, k * 128:(k + 1) * 128].rearrange("a b -> b a"))
                    nc.sync.dma_start(
                        wk_sb[:, k * 128:(k + 1) * 128],
                        wk_ext[:, k * 128:(k + 1) * 128].rearrange("a b -> b a"))
                    nc.sync.dma_start(
                        wv_sb[:, k * 128:(k + 1) * 128],
                        wv_ext[:, k * 128:(k + 1) * 128].rearrange("a b -> b a"))
                bq_sb = cpool.tile([W, 1], F32)
                nc.sync.dma_start(bq_sb[:], bq_ext[:])
                bk_sb = cpool.tile([W, 1], F32)
                nc.sync.dma_start(bk_sb[:], bk_ext[:])
                bv_sb = cpool.tile([W, 1], F32)
                nc.sync.dma_start(bv_sb[:], bv_ext[:])
                cos_sb = cpool.tile([W, L], F32R)
                nc.sync.dma_start(cos_sb[:], cos_ext[:])
                sin_sb = cpool.tile([W, L], F32R)
                nc.sync.dma_start(sin_sb[:], sin_ext[:])
                rmat_sb = cpool.tile([128, 128], F32R)
                nc.sync.dma_start(rmat_sb[:], rmat_ext[:])
                ident_sb = cpool.tile([128, 128], F32R)
                nc.sync.dma_start(ident_sb[:], ident_ext[:])
                mask_sb = cpool.tile([128, 2048], F32)
                nc.sync.dma_start(mask_sb[:], mask_ext[:])
                ones_sb = cpool.tile([128, 128], F32R)
                nc.sync.dma_start(ones_sb[:], ones_ext[:])

                o_sb = opool.tile([128, T], F32R)

                with tc.tile_pool(name="qkv", bufs=1) as qkpool:
                    q_ro = qkpool.tile([128, T], F32R)
                    k_ro = qkpool.tile([128, T], F32R)
                    v_sb = qkpool.tile([128, 32 * VBLK], F32R)
                    # ones columns of v (blocks at 64 + VBLK*j and 129 + VBLK*j)
                    v_view = v_sb[:].rearrange("p (b s) -> p b s", s=VBLK)
                    nc.sync.dma_start(v_view[:, :, 64], ones_ext[:, 0:32])
                    nc.sync.dma_start(v_view[:, :, 129], ones_ext[:, 32:64])

                    # ---------------- phase 1: QKV + RoPE ----------------
                    with tc.tile_pool(name="xt", bufs=16) as xtpool, \
                         tc.tile_pool(name="p1t", bufs=3) as t1pool, \
                         tc.tile_pool(name="p1ps", bufs=2, space="PSUM") as ps1, \
                         tc.tile_pool(name="p1vt", bufs=2, space="PSUM") as psvt:
                        for tt in range(TT):
                            xts = []
                            for k in range(KD):
                                xt = xtpool.tile([128, 512], F32R, name=f"xt{k}",
                                                 tag="xt")
                                nc.sync.dma_start(
                                    xt[:],
                                    xT_ext[k * 128:(k + 1) * 128,
                                           tt * 512:(tt + 1) * 512])
                                xts.append(xt)
                            lcol = (tt % (TT // B)) * 512  # rope table columns

                            for wsb, bsb, dst in ((wq_sb, bq_sb, q_ro),
                                                  (wk_sb, bk_sb, k_ro)):
                                ps = ps1.tile([128, 512], F32, tag="proj")
                                for k in range(KD):
                                    nc.tensor.matmul(
                                        ps[:], wsb[:, k * 128:(k + 1) * 128],
                                        xts[k][:], start=(k == 0),
                                        stop=(k == KD - 1))
                                bs = t1pool.tile([128, 512], F32R, tag="bs")
                                nc.scalar.activation(bs[:], ps[:], AF.Identity,
                                                     bias=bsb[:])
                                rot = ps1.tile([128, 512], F32, tag="proj")
                                nc.tensor.matmul(rot[:], rmat_sb[:], bs[:],
                                                 start=True, stop=True)
                                t1 = t1pool.tile([128, 512], F32, tag="t1")
                                nc.vector.tensor_mul(
                                    t1[:], bs[:], cos_sb[:, lcol:lcol + 512])
                                t2 = t1pool.tile([128, 512], F32, tag="t2")
                                nc.vector.tensor_mul(
                                    t2[:], rot[:], sin_sb[:, lcol:lcol + 512])
                                nc.vector.tensor_add(
                                    dst[:, tt * 512:(tt + 1) * 512], t1[:], t2[:])

                            # v: feature-major projection then PE transpose
                            ps = ps1.tile([128, 512], F32, tag="proj")
                            for k in range(KD):
                                nc.tensor.matmul(
                                    ps[:], wv_sb[:, k * 128:(k + 1) * 128],
                                    xts[k][:], start=(k == 0), stop=(k == KD - 1))
                            vbs = t1pool.tile([128, 512], F32R, tag="bs")
                            nc.scalar.activation(vbs[:], ps[:], AF.Identity,
                                                 bias=bv_sb[:])
                            for s in range(4):
                                vt = psvt.tile([128, 128], F32R, tag="vt")
                                nc.tensor.transpose(
                                    vt[:], vbs[:, s * 128:(s + 1) * 128],
                                    ident_sb[:])
                                blk = (tt * 4 + s) * VBLK
                                nc.vector.tensor_copy(
                                    v_sb[:, blk:blk + 64], vt[:, 0:64])
                                nc.vector.tensor_copy(
                                    v_sb[:, blk + 65:blk + 129], vt[:, 64:128])

                    # ---------------- phase 2: attention ----------------
                    with tc.tile_pool(name="att", bufs=4) as atpool, \
                         tc.tile_pool(name="nrm", bufs=2) as nrmpool, \
                         tc.tile_pool(name="psT", bufs=3, space="PSUM") as psT, \
                         tc.tile_pool(name="pso", bufs=2, space="PSUM") as pso, \
                         tc.tile_pool(name="psb", bufs=2, space="PSUM") as psb:
                        for b in range(B):
                            for hl in range(HPC):
                                hof = 64 * hl
                                tof = b * L
                                vb = b * (L // 128)
                                for qt in range(L // 512):
                                    nkc = 4 * qt + 4
                                    ops = pso.tile([65, 512], F32, tag="ops")
                                    for kc in range(nkc):
                                        sT = psT.tile([128, 512], F32, tag="sT")
                                        nc.tensor.matmul(
                                            sT[:],
                                            k_ro[hof:hof + 64,
                                                 tof + kc * 128:tof + kc * 128 + 128],
                                            q_ro[hof:hof + 64,
                                                 tof + qt * 512:tof + qt * 512 + 512],
                                            start=True, stop=True)
                                        r = kc - 4 * qt
                                        if r >= 0:
                                            nc.vector.tensor_add(
                                                sT[:], sT[:],
                                                mask_sb[:, r * 512:(r + 1) * 512])
                                        at = atpool.tile([128, 512], F32R, tag="at")
                                        nc.scalar.activation(at[:], sT[:], AF.Exp)
                                        nc.tensor.matmul(
                                            ops[:],
                                            v_sb[:, (vb + kc) * VBLK + 65 * hl:
                                                 (vb + kc) * VBLK + 65 * hl + 65],
                                            at[:], start=(kc == 0),
                                            stop=(kc == nkc - 1))
                                    rc = nrmpool.tile([1, 512], F32R, tag="rc")
                                    nc.vector.reciprocal(rc[:], ops[64:65, :])
                                    bc = psb.tile([64, 512], F32, tag="bc")
                                    nc.tensor.matmul(bc[:], ones_sb[0:1, 0:64],
                                                     rc[:], start=True, stop=True)
                                    bcs = nrmpool.tile([64, 512], F32R, tag="bcs")
                                    nc.scalar.activation(bcs[:], bc[:], AF.Identity)
                                    nc.vector.tensor_mul(
                                        o_sb[hof:hof + 64,
                                             tof + qt * 512:tof + qt * 512 + 512],
                                        ops[0:64, :], bcs[:])

                # ---------------- phase 3: AllToAll ----------------
                for j in range(NC):
                    nc.sync.dma_start(cin[j], o_sb[:, j * 512:(j + 1) * 512])
                nc.gpsimd.collective_compute(
                    "AllToAll", mybir.AluOpType.bypass,
                    replica_groups=[list(range(NC))],
                    ins=[cin[:].opt()], outs=[cout[:].opt()],
                )

                # ---------------- phase 4: output projection ----------------
                with tc.tile_pool(name="og", bufs=1) as ogpool, \
                     tc.tile_pool(name="wo", bufs=1) as wopool, \
                     tc.tile_pool(name="ob", bufs=4) as obpool, \
                     tc.tile_pool(name="ps4", bufs=4, space="PSUM") as ps4:
                    ogs = []
                    for k in range(NC):
                        og = ogpool.tile([128, 512], F32R, name=f"og{k}", tag=f"og{k}")
                        nc.sync.dma_start(og[:], cout[k])
                        ogs.append(og)
                    wo_sb = wopool.tile([128, NC * D], F32R)
                    nc.sync.dma_start(wo_sb[:], wo_ext[:])
                    bo_sb = wopool.tile([1, D], F32R)
                    nc.sync.dma_start(bo_sb[:], bo_ext[:])

                    for tb in range(4):
                        for half in range(2):
                            ps = ps4.tile([128, 512], F32, tag="op")
                            for k in range(NC):
                                nc.tensor.matmul(
                                    ps[:], ogs[k][:, tb * 128:(tb + 1) * 128],
                                    wo_sb[:, k * D + half * 512:
                                          k * D + half * 512 + 512],
                                    start=(k == 0), stop=False)
                            nc.tensor.matmul(
                                ps[:], ones_sb[0:1, :],
                                bo_sb[:, half * 512:half * 512 + 512],
                                start=False, stop=True)
                            ob = obpool.tile([128, 512], F32, tag="ob")
                            nc.scalar.activation(ob[:], ps[:], AF.Identity)
                            nc.sync.dma_start(
                                out_ext[tb * 128:(tb + 1) * 128,
                                        half * 512:half * 512 + 512], ob[:])

    # legalize: never more than 2 sync commands (waits+updates) per instruction
    import bass_rust
    from concourse import mybir as _mb
    uid = [0]
    for bb in nc.m.functions[0].blocks:
        il = bb.instructions
        todo = [i for i, inst in enumerate(il)
                if inst.sync_info is not None
                and len(inst.sync_info.on_wait) + len(inst.sync_info.on_update) > 2]
        for idx in reversed(todo):
            inst = il[idx]
            si = inst.sync_info
            waits = list(si.on_wait)
            n_keep = max(0, 2 - len(si.on_update))
            keep = waits[len(waits) - n_keep:] if n_keep else []
            excess = waits[: len(waits) - n_keep]
            nops = []
            for i in range(0, len(excess)):
                uid[0] += 1
                nops.append(_mb.InstNoOp(
                    name=f"WSPLIT-{uid[0]}", engine=inst.engine, ins=[], outs=[],
                    bass_nofuse=True,
                    sync_info=bass_rust.SyncInfo(on_wait=excess[i:i + 1],
                                                 on_update=[])))
            inst.sync_info = bass_rust.SyncInfo(on_wait=keep,
                                                on_update=list(si.on_update))
            for j, nop in enumerate(nops):
                il.insert(idx + j, nop)
    return nc


def _host_prep(x, Wq, bq, Wk, bk, Wv, bv, Wo, bo, scale):
    s = float(np.asarray(scale).reshape(-1)[0])
    xT = np.ascontiguousarray(x.reshape(T, D).T.astype(np.float32))

    # RoPE tables, feature-major, rows duplicated per interleaved pair
    freqs = THETA ** (-np.arange(0, H, 2, dtype=np.float64) / H)      # [32]
    ang = np.arange(L, dtype=np.float64)[:, None] * freqs[None, :]    # [L, 32]
    cos_t = np.repeat(np.cos(ang).T, 2, axis=0)                       # [64, L]
    sin_t = np.repeat(np.sin(ang).T, 2, axis=0)
    cosT = np.ascontiguousarray(np.tile(cos_t, (HPC, 1)).astype(np.float32))
    sinT = np.ascontiguousarray(np.tile(sin_t, (HPC, 1)).astype(np.float32))

    rmat = np.zeros((128, 128), dtype=np.float32)
    for i in range(64):
        rmat[2 * i + 1, 2 * i] = -1.0
        rmat[2 * i, 2 * i + 1] = 1.0

    ident = np.eye(128, dtype=np.float32)
    onesc = np.ones((128, 128), dtype=np.float32)

    kt = np.arange(128)[:, None]
    qc = np.arange(512)[None, :]
    masks = np.concatenate(
        [np.where(128 * r + kt <= qc, 0.0, NEG).astype(np.float32)
         for r in range(4)], axis=1)                                   # [128, 2048]

    woT = np.ascontiguousarray(
        Wo.T.astype(np.float32).reshape(NC, 128, D).transpose(1, 0, 2)
        .reshape(128, NC * D))
    bo_row = np.ascontiguousarray(bo.astype(np.float32).reshape(1, D))

    Wq_s = (Wq * s).astype(np.float32)
    bq_s = (bq * s).astype(np.float32)

    in_maps = []
    for c in range(NC):
        hsl = slice(c * W, (c + 1) * W)
        in_maps.append({
            "xT": xT,
            "wq": np.ascontiguousarray(Wq_s[hsl, :]),
            "wk": np.ascontiguousarray(Wk[hsl, :].astype(np.float32)),
            "wv": np.ascontiguousarray(Wv[hsl, :].astype(np.float32)),
            "bq": np.ascontiguousarray(bq_s[hsl].reshape(W, 1)),
            "bk": np.ascontiguousarray(bk[hsl].astype(np.float32).reshape(W, 1)),
            "bv": np.ascontiguousarray(bv[hsl].astype(np.float32).reshape(W, 1)),
            "cosT": cosT, "sinT": sinT, "rmat": rmat, "ident": ident,
            "masks": masks, "onesc": onesc, "woT": woT, "bo": bo_row,
        })
    return in_maps


def kernel(x, Wq, bq, Wk, bk, Wv, bv, Wo, bo, scale):
    from concourse.bass_utils import run_bass_kernel_spmd

    if "nc" not in _CACHED:
        _CACHED["nc"] = _build()
    nc = _CACHED["nc"]
    in_maps = _host_prep(np.asarray(x), np.asarray(Wq), np.asarray(bq),
                         np.asarray(Wk), np.asarray(bk), np.asarray(Wv),
                         np.asarray(bv), np.asarray(Wo), np.asarray(bo),
                         np.asarray(scale))
    res = run_bass_kernel_spmd(nc, in_maps, list(range(NC)))
    out = np.concatenate([res.results[c]["out"] for c in range(NC)], axis=0)
    return out.reshape(B, L, D).astype(np.float32)


# revision 3
# speedup vs baseline: 2.0127x; 2.0127x over previous
"""Causal multi-head attention (RoPE) on 8 TRN2 NeuronCores.

Sharding: Megatron-style head parallelism. Each core owns 2 of the 16 heads:
it computes q/k/v projections for its 128 output features (2 heads x 64),
applies interleaved-pair RoPE (rotation done as a PE matmul with a constant
pair-swap matrix R, tables precomputed host-side), runs causal attention for
its (batch, head) pairs in the transposed orientation S^T = K^T Q so that no
on-chip transposes of the attention matrix are needed, and accumulates
attn^T-weighted V with an extra all-ones V column that yields the softmax
denominators for free. An AllToAll then redistributes the per-head outputs
from head-sharded to token-sharded layout, and each core computes the final
output projection for its 512-token slice. A tiny dummy AllToAll is issued at
kernel start to absorb the one-time collective warmup (~95us) while the
engines compute.

Compute dtype: float32r (TF32-like full-rate fp32 matmul path; ~2^-11
element rounding, accumulation in fp32 PSUM).
"""

import sys

sys.path.insert(0, "/opt/trn_rl_repo")

import numpy as np

B, L, D, N, H = 2, 2048, 1024, 16, 64
T = B * L            # 4096 tokens, batch-major
NC = 8               # cores
HPC = N // NC        # 2 heads per core
W = HPC * H          # 128 projection features per core
THETA = 10000.0
VBLK = 130           # v_sb block stride: [h0 64 | ones 1 | h1 64 | ones 1]
NEG = -60.0          # additive causal mask value (exp(-60) ~ 9e-27)

_CACHED = {}


def _build():
    import concourse.bass as bass
    import concourse.mybir as mybir
    import concourse.tile as tile

    F32 = mybir.dt.float32
    F32R = mybir.dt.float32r
    AF = mybir.ActivationFunctionType

    # ---- fix 1: stock _drain_and_barrier overflows the 2-slot sync encoding
    import re as _re
    from concourse.vector_clock import ScopedClock, VectorClock

    def _split_drain_and_barrier(self, tick_clock, wait_clock):
        gc = tick_clock.global_clock
        ticks = [int(v) for v in _re.findall(r"-?\d+", str(gc))]
        for proc, t in enumerate(ticks):
            if t <= 0:
                continue
            sub = VectorClock()
            sub.require_at_least(proc, t)
            d = self.nc.sync.drain()
            wait_clock.add_sem_waits(d.ins, ScopedClock({None: sub}))
        self.nc.all_engine_barrier()
        assert self.sems is not None
        popped = self.nc._tile_sem_poison_stack.pop()
        assert popped is self._sem_poison
        self.nc.clear_and_free_semaphores(list(self.sems.allocated().values()))
        self.nc.all_engine_barrier()

    tile.TileContext._drain_and_barrier = _split_drain_and_barrier

    nc = bass.Bass()

    xT_ext = nc.declare_dram_parameter("xT", [D, T], F32R, isOutput=False)
    wq_ext = nc.declare_dram_parameter("wq", [D, W], F32R, isOutput=False)
    wk_ext = nc.declare_dram_parameter("wk", [D, W], F32R, isOutput=False)
    wv_ext = nc.declare_dram_parameter("wv", [D, W], F32R, isOutput=False)
    bq_ext = nc.declare_dram_parameter("bq", [W, 1], F32, isOutput=False)
    bk_ext = nc.declare_dram_parameter("bk", [W, 1], F32, isOutput=False)
    bv_ext = nc.declare_dram_parameter("bv", [W, 1], F32, isOutput=False)
    cos_ext = nc.declare_dram_parameter("cosT", [W, L], F32R, isOutput=False)
    sin_ext = nc.declare_dram_parameter("sinT", [W, L], F32R, isOutput=False)
    rmat_ext = nc.declare_dram_parameter("rmat", [128, 128], F32R, isOutput=False)
    ident_ext = nc.declare_dram_parameter("ident", [128, 128], F32R, isOutput=False)
    mask_ext = nc.declare_dram_parameter("masks", [128, 2048], F32, isOutput=False)
    ones_ext = nc.declare_dram_parameter("onesc", [128, 128], F32R, isOutput=False)
    wo_ext = nc.declare_dram_parameter("woT", [128, NC * D], F32R, isOutput=False)
    bo_ext = nc.declare_dram_parameter("bo", [1, D], F32R, isOutput=False)
    out_ext = nc.declare_dram_parameter("out", [T // NC, D], F32, isOutput=True)

    TT = T // 512      # 8 token tiles of 512
    KD = D // 128      # 8 contraction chunks

    with tile.TileContext(nc) as tc, nc.allow_low_precision(reason="f32r attention"):
        with tc.tile_pool(name="dram", bufs=1, space="DRAM") as dram:
            # dummy collective: absorbs the one-time collective warmup cost
            # while phase 1/2 compute runs. Data content is irrelevant.
            dum_in = dram.tile([NC, 128], F32)
            dum_out = dram.tile([NC, 128], F32)
            nc.gpsimd.collective_compute(
                "AllToAll", mybir.AluOpType.bypass,
                replica_groups=[list(range(NC))],
                ins=[dum_in[:].opt()], outs=[dum_out[:].opt()],
            )

            cin = dram.tile([NC, 128, 512], F32R)
            cout = dram.tile([NC, 128, 512], F32R)

            with tc.tile_pool(name="const", bufs=1) as cpool, \
                 tc.tile_pool(name="obuf", bufs=1) as opool:
                # constants
                wq_sb = cpool.tile([128, KD * 128], F32R)
                wk_sb = cpool.tile([128, KD * 128], F32R)
                wv_sb = cpool.tile([128, KD * 128], F32R)
                for k in range(KD):
                    # host passes w.T [D, W]; lhsT chunk k = wT[k*128:(k+1)*128, :]
                    nc.sync.dma_start(wq_sb[:, k * 128:(k + 1) * 128],
                                      wq_ext[k * 128:(k + 1) * 128, :])
                    nc.sync.dma_start(wk_sb[:, k * 128:(k + 1) * 128],
                                      wk_ext[k * 128:(k + 1) * 128, :])
                    nc.sync.dma_start(wv_sb[:, k * 128:(k + 1) * 128],
                                      wv_ext[k * 128:(k + 1) * 128, :])
                bq_sb = cpool.tile([W, 1], F32)
                nc.sync.dma_start(bq_sb[:], bq_ext[:])
                bk_sb = cpool.tile([W, 1], F32)
                nc.sync.dma_start(bk_sb[:], bk_ext[:])
                bv_sb = cpool.tile([W, 1], F32)
                nc.sync.dma_start(bv_sb[:], bv_ext[:])
                cos_sb = cpool.tile([W, L], F32R)
                nc.sync.dma_start(cos_sb[:], cos_ext[:])
                sin_sb = cpool.tile([W, L], F32R)
                nc.sync.dma_start(sin_sb[:], sin_ext[:])
                rmat_sb = cpool.tile([128, 128], F32R)
                nc.sync.dma_start(rmat_sb[:], rmat_ext[:])
                ident_sb = cpool.tile([128, 128], F32R)
                nc.sync.dma_start(ident_sb[:], ident_ext[:])
                mask_sb = cpool.tile([128, 2048], F32)
                nc.sync.dma_start(mask_sb[:], mask_ext[:])
                ones_sb = cpool.tile([128, 128], F32R)
                nc.sync.dma_start(ones_sb[:], ones_ext[:])

                o_sb = opool.tile([128, T], F32R)

                with tc.tile_pool(name="qkv", bufs=1) as qkpool:
                    q_ro = qkpool.tile([128, T], F32R)
                    k_ro = qkpool.tile([128, T], F32R)
                    v_sb = qkpool.tile([128, 32 * VBLK], F32R)
                    # ones columns of v (blocks at 64 + VBLK*j and 129 + VBLK*j)
                    v_view = v_sb[:].rearrange("p (b s) -> p b s", s=VBLK)
                    nc.sync.dma_start(v_view[:, :, 64], ones_ext[:, 0:32])
                    nc.sync.dma_start(v_view[:, :, 129], ones_ext[:, 32:64])

                    # ---------------- phase 1: QKV + RoPE ----------------
                    with tc.tile_pool(name="xt", bufs=16) as xtpool, \
                         tc.tile_pool(name="p1t", bufs=3) as t1pool, \
                         tc.tile_pool(name="p1ps", bufs=2, space="PSUM") as ps1, \
                         tc.tile_pool(name="p1vt", bufs=2, space="PSUM") as psvt:
                        for tt in range(TT):
                            xts = []
                            for k in range(KD):
                                xt = xtpool.tile([128, 512], F32R, name=f"xt{k}",
                                                 tag="xt")
                                nc.sync.dma_start(
                                    xt[:],
                                    xT_ext[k * 128:(k + 1) * 128,
                                           tt * 512:(tt + 1) * 512])
                                xts.append(xt)
                            lcol = (tt % (TT // B)) * 512  # rope table columns

                            for wsb, bsb, dst in ((wq_sb, bq_sb, q_ro),
                                                  (wk_sb, bk_sb, k_ro)):
                                ps = ps1.tile([128, 512], F32, tag="proj")
                                for k in range(KD):
                                    nc.tensor.matmul(
                                        ps[:], wsb[:, k * 128:(k + 1) * 128],
                                        xts[k][:], start=(k == 0),
                                        stop=(k == KD - 1))
                                bs = t1pool.tile([128, 512], F32R, tag="bs")
                                nc.scalar.activation(bs[:], ps[:], AF.Identity,
                                                     bias=bsb[:])
                                rot = ps1.tile([128, 512], F32, tag="proj")
                                nc.tensor.matmul(rot[:], rmat_sb[:], bs[:],
                                                 start=True, stop=True)
                                t1 = t1pool.tile([128, 512], F32, tag="t1")
                                nc.vector.tensor_mul(
                                    t1[:], bs[:], cos_sb[:, lcol:lcol + 512])
                                t2 = t1pool.tile([128, 512], F32, tag="t2")
                                nc.vector.tensor_mul(
                                    t2[:], rot[:], sin_sb[:, lcol:lcol + 512])
                                nc.vector.tensor_add(
                                    dst[:, tt * 512:(tt + 1) * 512], t1[:], t2[:])

                            # v: feature-major projection then PE transpose
                            ps = ps1.tile([128, 512], F32, tag="proj")
                            for k in range(KD):
                                nc.tensor.matmul(
                                    ps[:], wv_sb[:, k * 128:(k + 1) * 128],
                                    xts[k][:], start=(k == 0), stop=(k == KD - 1))
                            vbs = t1pool.tile([128, 512], F32R, tag="bs")
                            nc.scalar.activation(vbs[:], ps[:], AF.Identity,
                                                 bias=bv_sb[:])
                            for s in range(4):
                                vt = psvt.tile([128, 128], F32R, tag="vt")
                                nc.tensor.transpose(
                                    vt[:], vbs[:, s * 128:(s + 1) * 128],
                                    ident_sb[:])
                                blk = (tt * 4 + s) * VBLK
                                nc.vector.tensor_copy(
                                    v_sb[:, blk:blk + 64], vt[:, 0:64])
                                nc.vector.tensor_copy(
                                    v_sb[:, blk + 65:blk + 129], vt[:, 64:128])

                    # ---------------- phase 2: attention ----------------
                    with tc.tile_pool(name="att", bufs=4) as atpool, \
                         tc.tile_pool(name="nrm", bufs=2) as nrmpool, \
                         tc.tile_pool(name="psT", bufs=3, space="PSUM") as psT, \
                         tc.tile_pool(name="pso", bufs=2, space="PSUM") as pso, \
                         tc.tile_pool(name="psb", bufs=2, space="PSUM") as psb:
                        for b in range(B):
                            for hl in range(HPC):
                                hof = 64 * hl
                                tof = b * L
                                vb = b * (L // 128)
                                for qt in range(L // 512):
                                    nkc = 4 * qt + 4
                                    ops = pso.tile([65, 512], F32, tag="ops")
                                    for kc in range(nkc):
                                        sT = psT.tile([128, 512], F32, tag="sT")
                                        nc.tensor.matmul(
                                            sT[:],
                                            k_ro[hof:hof + 64,
                                                 tof + kc * 128:tof + kc * 128 + 128],
                                            q_ro[hof:hof + 64,
                                                 tof + qt * 512:tof + qt * 512 + 512],
                                            start=True, stop=True)
                                        r = kc - 4 * qt
                                        if r >= 0:
                                            nc.vector.tensor_add(
                                                sT[:], sT[:],
                                                mask_sb[:, r * 512:(r + 1) * 512])
                                        at = atpool.tile([128, 512], F32R, tag="at")
                                        nc.scalar.activation(at[:], sT[:], AF.Exp)
                                        nc.tensor.matmul(
                                            ops[:],
                                            v_sb[:, (vb + kc) * VBLK + 65 * hl:
                                                 (vb + kc) * VBLK + 65 * hl + 65],
                                            at[:], start=(kc == 0),
                                            stop=(kc == nkc - 1))
                                    rc = nrmpool.tile([1, 512], F32R, tag="rc")
                                    nc.vector.reciprocal(rc[:], ops[64:65, :])
                                    bc = psb.tile([64, 512], F32, tag="bc")
                                    nc.tensor.matmul(bc[:], ones_sb[0:1, 0:64],
                                                     rc[:], start=True, stop=True)
                                    bcs = nrmpool.tile([64, 512], F32R, tag="bcs")
                                    nc.scalar.activation(bcs[:], bc[:], AF.Identity)
                                    nc.vector.tensor_mul(
                                        o_sb[hof:hof + 64,
                                             tof + qt * 512:tof + qt * 512 + 512],
                                        ops[0:64, :], bcs[:])

                # ---------------- phase 3: AllToAll ----------------
                for j in range(NC):
                    nc.sync.dma_start(cin[j], o_sb[:, j * 512:(j + 1) * 512])
                nc.gpsimd.collective_compute(
                    "AllToAll", mybir.AluOpType.bypass,
                    replica_groups=[list(range(NC))],
                    ins=[cin[:].opt()], outs=[cout[:].opt()],
                )

                # ---------------- phase 4: output projection ----------------
                with tc.tile_pool(name="og", bufs=1) as ogpool, \
                     tc.tile_pool(name="wo", bufs=1) as wopool, \
                     tc.tile_pool(name="ob", bufs=4) as obpool, \
                     tc.tile_pool(name="ps4", bufs=4, space="PSUM") as ps4:
                    ogs = []
                    for k in range(NC):
                        og = ogpool.tile([128, 512], F32R, name=f"og{k}", tag=f"og{k}")
                        nc.sync.dma_start(og[:], cout[k])
                        ogs.append(og)
                    wo_sb = wopool.tile([128, NC * D], F32R)
                    nc.sync.dma_start(wo_sb[:], wo_ext[:])
                    bo_sb = wopool.tile([1, D], F32R)
                    nc.sync.dma_start(bo_sb[:], bo_ext[:])

                    for tb in range(4):
                        for half in range(2):
                            ps = ps4.tile([128, 512], F32, tag="op")
                            for k in range(NC):
                                nc.tensor.matmul(
                                    ps[:], ogs[k][:, tb * 128:(tb + 1) * 128],
                                    wo_sb[:, k * D + half * 512:
                                          k * D + half * 512 + 512],
                                    start=(k == 0), stop=False)
                            nc.tensor.matmul(
                                ps[:], ones_sb[0:1, :],
                                bo_sb[:, half * 512:half * 512 + 512],
                                start=False, stop=True)
                            ob = obpool.tile([128, 512], F32, tag="ob")
                            nc.scalar.activation(ob[:], ps[:], AF.Identity)
                            nc.sync.dma_start(
                                out_ext[tb * 128:(tb + 1) * 128,
                                        half * 512:half * 512 + 512], ob[:])

    # legalize: never more than 2 sync commands (waits+updates) per instruction
    import bass_rust
    from concourse import mybir as _mb
    uid = [0]
    for bb in nc.m.functions[0].blocks:
        il = bb.instructions
        todo = [i for i, inst in enumerate(il)
                if inst.sync_info is not None
                and len(inst.sync_info.on_wait) + len(inst.sync_info.on_update) > 2]
        for idx in reversed(todo):
            inst = il[idx]
            si = inst.sync_info
            waits = list(si.on_wait)
            n_keep = max(0, 2 - len(si.on_update))
            keep = waits[len(waits) - n_keep:] if n_keep else []
            excess = waits[: len(waits) - n_keep]
            nops = []
            for i in range(0, len(excess)):
                uid[0] += 1
                nops.append(_mb.InstNoOp(
                    name=f"WSPLIT-{uid[0]}", engine=inst.engine, ins=[], outs=[],
                    bass_nofuse=True,
                    sync_info=bass_rust.SyncInfo(on_wait=excess[i:i + 1],
                                                 on_update=[])))
            inst.sync_info = bass_rust.SyncInfo(on_wait=keep,
                                                on_update=list(si.on_update))
            for j, nop in enumerate(nops):
                il.insert(idx + j, nop)
    return nc


def _host_prep(x, Wq, bq, Wk, bk, Wv, bv, Wo, bo, scale):
    s = float(np.asarray(scale).reshape(-1)[0])
    xT = np.ascontiguousarray(x.reshape(T, D).T.astype(np.float32))

    # RoPE tables, feature-major, rows duplicated per interleaved pair
    freqs = THETA ** (-np.arange(0, H, 2, dtype=np.float64) / H)      # [32]
    ang = np.arange(L, dtype=np.float64)[:, None] * freqs[None, :]    # [L, 32]
    cos_t = np.repeat(np.cos(ang).T, 2, axis=0)                       # [64, L]
    sin_t = np.repeat(np.sin(ang).T, 2, axis=0)
    cosT = np.ascontiguousarray(np.tile(cos_t, (HPC, 1)).astype(np.float32))
    sinT = np.ascontiguousarray(np.tile(sin_t, (HPC, 1)).astype(np.float32))

    rmat = np.zeros((128, 128), dtype=np.float32)
    for i in range(64):
        rmat[2 * i + 1, 2 * i] = -1.0
        rmat[2 * i, 2 * i + 1] = 1.0

    ident = np.eye(128, dtype=np.float32)
    onesc = np.ones((128, 128), dtype=np.float32)

    kt = np.arange(128)[:, None]
    qc = np.arange(512)[None, :]
    masks = np.concatenate(
        [np.where(128 * r + kt <= qc, 0.0, NEG).astype(np.float32)
         for r in range(4)], axis=1)                                   # [128, 2048]

    woT = np.ascontiguousarray(
        Wo.T.astype(np.float32).reshape(NC, 128, D).transpose(1, 0, 2)
        .reshape(128, NC * D))
    bo_row = np.ascontiguousarray(bo.astype(np.float32).reshape(1, D))

    Wq_s = (Wq * s).astype(np.float32)
    bq_s = (bq * s).astype(np.float32)

    in_maps = []
    for c in range(NC):
        hsl = slice(c * W, (c + 1) * W)
        in_maps.append({
            "xT": xT,
            "wq": np.ascontiguousarray(Wq_s[hsl, :].T),
            "wk": np.ascontiguousarray(Wk[hsl, :].astype(np.float32).T),
            "wv": np.ascontiguousarray(Wv[hsl, :].astype(np.float32).T),
            "bq": np.ascontiguousarray(bq_s[hsl].reshape(W, 1)),
            "bk": np.ascontiguousarray(bk[hsl].astype(np.float32).reshape(W, 1)),
            "bv": np.ascontiguousarray(bv[hsl].astype(np.float32).reshape(W, 1)),
            "cosT": cosT, "sinT": sinT, "rmat": rmat, "ident": ident,
            "masks": masks, "onesc": onesc, "woT": woT, "bo": bo_row,
        })
    return in_maps


def kernel(x, Wq, bq, Wk, bk, Wv, bv, Wo, bo, scale):
    from concourse.bass_utils import run_bass_kernel_spmd

    if "nc" not in _CACHED:
        _CACHED["nc"] = _build()
    nc = _CACHED["nc"]
    in_maps = _host_prep(np.asarray(x), np.asarray(Wq), np.asarray(bq),
                         np.asarray(Wk), np.asarray(bk), np.asarray(Wv),
                         np.asarray(bv), np.asarray(Wo), np.asarray(bo),
                         np.asarray(scale))
    res = run_bass_kernel_spmd(nc, in_maps, list(range(NC)))
    out = np.concatenate([res.results[c]["out"] for c in range(NC)], axis=0)
    return out.reshape(B, L, D).astype(np.float32)
